# revision 1
# baseline (speedup 1.0000x reference)
"""L2-similarity kernel for Trainium2 (8 NeuronCores, SPMD).

Computes logits[n, m] = -sqrt(max(||x_n||^2 + ||g_m||^2 - 2 x_n.g_m, 0))
for x = image_features [8192, 128], g = gts [16384, 128].

Sharding: gallery-parallel — each core owns a [2048, 128] slice of gts and
computes the full [8192, 2048] column block; blocks concat along axis 1.

Per core: PE computes -2*x@g^T (float32r, full rate) plus a rank-1 augment
that adds ||g_m||^2; ACT applies Sqrt with per-partition bias ||x_n||^2;
DVE negates; output DMAs are fully contiguous 1 MiB row blocks.
"""
import sys

if "/opt/trn_rl_repo" not in sys.path:
    sys.path.insert(0, "/opt/trn_rl_repo")

import numpy as np

P = 128
N = 8192          # image rows (replicated on every core)
M_FULL = 16384    # gallery rows (sharded)
D = 128
NCORES = 8
M = M_FULL // NCORES   # 2048 gallery rows per core
JT = 512               # psum free-dim tile
XG = 8                 # x chunks per load group

_nc_cache = None


def _build():
    import concourse.mybir as mybir
    from concourse import bacc
    from concourse.tile import TileContext
    from concourse.masks import make_identity

    F32 = mybir.dt.float32
    F32R = mybir.dt.float32r
    AF = mybir.ActivationFunctionType

    nc = bacc.Bacc("TRN2", target_bir_lowering=False)
    x = nc.dram_tensor("x", [N, D], F32, kind="ExternalInput")
    g = nc.dram_tensor("g", [M, D], F32, kind="ExternalInput")
    out = nc.dram_tensor("out", [N, M], F32, kind="ExternalOutput")

    n_chunks = N // P       # 64
    m_tiles = M // P        # 16
    j_tiles = M // JT       # 4

    with TileContext(nc) as tc:
        with (
            tc.tile_pool(name="const", bufs=1) as const,
            tc.tile_pool(name="sbuf", bufs=2) as sbuf,
            tc.tile_pool(name="tpsum", bufs=2, space="PSUM") as tpsum,
            tc.tile_pool(name="mmpsum", bufs=6, space="PSUM") as mmpsum,
            tc.tile_pool(name="stage", bufs=4) as stagep,
        ):
            ident = const.tile([P, P], F32)
            make_identity(nc, ident)

            xTm2 = const.tile([P, N], F32R)       # -2 * x^T
            gT = const.tile([P, M], F32R)         # g^T
            x2c = const.tile([P, n_chunks], F32)  # ||x||^2, col per chunk
            g2c = const.tile([P, m_tiles], F32)   # ||g||^2, col per m-tile
            g2row_f = const.tile([1, M], F32)
            g2row = const.tile([1, M], F32R)
            ones1 = const.tile([1, P], F32R)
            ones1_f = const.tile([1, P], F32)

            nc.vector.memset(ones1_f, 1.0)
            nc.vector.tensor_copy(out=ones1, in_=ones1_f)

            # ---- gallery shard: load, transpose, row squares ----
            gld = sbuf.tile([P, m_tiles, D], F32, tag="gload")
            nc.sync.dma_start(out=gld, in_=g.rearrange("(t p) d -> p t d", p=P))
            for t in range(m_tiles):
                pt = tpsum.tile([P, P], F32, tag="tp")
                nc.tensor.transpose(pt, gld[:, t, :], ident)
                nc.vector.tensor_copy(out=gT[:, t * P:(t + 1) * P], in_=pt)
                sqd = sbuf.tile([P, D], F32, tag="sqdump")
                nc.scalar.activation(sqd, gld[:, t, :], AF.Square,
                                     accum_out=g2c[:, t:t + 1])
            for t in range(m_tiles):
                nc.sync.dma_start(out=g2row_f[0:1, t * P:(t + 1) * P],
                                  in_=g2c[:, t:t + 1])
            nc.vector.tensor_copy(out=g2row, in_=g2row_f)

            # ---- main loop: one 128-row block of x at a time ----
            for i in range(n_chunks):
                gi, qi = divmod(i, XG)
                if qi == 0:
                    xld = sbuf.tile([P, XG, D], F32, tag="xload")
                    nc.sync.dma_start(
                        out=xld,
                        in_=x[gi * XG * P:(gi + 1) * XG * P, :]
                            .rearrange("(q p) d -> p q d", p=P),
                    )
                pt = tpsum.tile([P, P], F32, tag="tp")
                nc.tensor.transpose(pt, xld[:, qi, :], ident)
                nc.scalar.activation(xTm2[:, i * P:(i + 1) * P], pt,
                                     AF.Copy, scale=-2.0)
                sqd = sbuf.tile([P, D], F32, tag="sqdump")
                nc.scalar.activation(sqd, xld[:, qi, :], AF.Square,
                                     accum_out=x2c[:, i:i + 1])

                stage = stagep.tile([P, M], F32, tag="stage")
                for j in range(j_tiles):
                    pm = mmpsum.tile([P, JT], F32, tag="mm")
                    nc.tensor.matmul(
                        pm,
                        lhsT=xTm2[:, i * P:(i + 1) * P],
                        rhs=gT[:, j * JT:(j + 1) * JT],
                        start=True, stop=False,
                    )
                    nc.tensor.matmul(
                        pm,
                        lhsT=ones1,
                        rhs=g2row[:, j * JT:(j + 1) * JT],
                        start=False, stop=True,
                    )
                    nc.scalar.activation(stage[:, j * JT:(j + 1) * JT], pm,
                                         AF.Sqrt, bias=x2c[:, i:i + 1])
                nc.vector.tensor_scalar_mul(stage, stage, -1.0)
                nc.sync.dma_start(out=out[i * P:(i + 1) * P, :], in_=stage)
    nc.finalize()
    return nc


def _get_nc():
    global _nc_cache
    if _nc_cache is None:
        _nc_cache = _build()
    return _nc_cache


def run_spmd(x_np, g_np, trace=False):
    """Run on 8 cores; returns (list of per-core out blocks, BassKernelResults)."""
    from concourse.bass_utils import run_bass_kernel_spmd

    in_maps = [
        {"x": x_np, "g": g_np[c * M:(c + 1) * M]} for c in range(NCORES)
    ]
    res = run_bass_kernel_spmd(_get_nc(), in_maps,
                               core_ids=list(range(NCORES)), trace=trace)
    blocks = [res.results[c]["out"] for c in range(NCORES)]
    return blocks, res


def kernel(image_features, gts):
    x_np = np.ascontiguousarray(np.asarray(image_features, dtype=np.float32))
    g_np = np.ascontiguousarray(np.asarray(gts, dtype=np.float32))
    assert x_np.shape == (N, D) and g_np.shape == (M_FULL, D)
    blocks, _ = run_spmd(x_np, g_np, trace=False)
    return np.concatenate(blocks, axis=1)


# revision 2
# speedup vs baseline: 1.5439x; 1.5439x over previous
"""L2-similarity kernel for Trainium2 (8 NeuronCores, SPMD).

Computes logits[n, m] = -sqrt(max(||x_n||^2 + ||g_m||^2 - 2 x_n.g_m, 0))
for x = image_features [8192, 128], g = gts [16384, 128].

Sharding: gallery-parallel — each core owns a [2048, 128] slice of gts and
computes the full [8192, 2048] column block; blocks concat along axis 1.

Per core: PE computes -2*x@g^T (float32r, full rate) plus a rank-1 augment
that adds ||g_m||^2; ACT applies Sqrt with per-partition bias ||x_n||^2;
DVE negates; output DMAs are fully contiguous 1 MiB row blocks.
"""
import sys

if "/opt/trn_rl_repo" not in sys.path:
    sys.path.insert(0, "/opt/trn_rl_repo")

import numpy as np

P = 128
N = 8192          # image rows (replicated on every core)
M_FULL = 16384    # gallery rows (sharded)
D = 128
NCORES = 8
M = M_FULL // NCORES   # 2048 gallery rows per core
JT = 512               # psum free-dim tile
XG = 8                 # x chunks per load group

_nc_cache = None


def _build():
    import concourse.mybir as mybir
    from concourse import bacc
    from concourse.tile import TileContext
    from concourse.masks import make_identity

    F32 = mybir.dt.float32
    BF16 = mybir.dt.bfloat16
    AF = mybir.ActivationFunctionType

    nc = bacc.Bacc("TRN2", target_bir_lowering=False)
    x = nc.dram_tensor("x", [N, D], F32, kind="ExternalInput")
    g = nc.dram_tensor("g", [M, D], F32, kind="ExternalInput")
    out = nc.dram_tensor("out", [N, M], F32, kind="ExternalOutput")

    n_chunks = N // P       # 64
    m_tiles = M // P        # 16
    j_tiles = M // JT       # 4

    with TileContext(nc) as tc:
        with (
            tc.tile_pool(name="const", bufs=1) as const,
            tc.tile_pool(name="sbuf", bufs=2) as sbuf,
            tc.tile_pool(name="tpsum", bufs=2, space="PSUM") as tpsum,
            tc.tile_pool(name="mmpsum", bufs=6, space="PSUM") as mmpsum,
            tc.tile_pool(name="stage", bufs=4) as stagep,
        ):
            ident = const.tile([P, P], F32)
            make_identity(nc, ident)

            xTm2 = const.tile([P, N], BF16)       # -2 * x^T
            gT = const.tile([P, M], BF16)         # g^T
            x2c = const.tile([P, n_chunks], F32)  # ||x||^2, col per chunk
            g2c = const.tile([P, m_tiles], F32)   # ||g||^2, col per m-tile
            g2row_f = const.tile([1, M], F32)
            g2row = const.tile([1, M], BF16)
            ones1 = const.tile([1, P], BF16)
            nc.vector.memset(ones1, 1.0)

            # ---- gallery shard: load, transpose, row squares ----
            gld = sbuf.tile([P, m_tiles, D], F32, tag="gload")
            nc.sync.dma_start(out=gld, in_=g.rearrange("(t p) d -> p t d", p=P))
            for t in range(m_tiles):
                pt = tpsum.tile([P, P], F32, tag="tp")
                nc.tensor.transpose(pt, gld[:, t, :], ident)
                nc.vector.tensor_copy(out=gT[:, t * P:(t + 1) * P], in_=pt)
                sqd = sbuf.tile([P, D], F32, tag="sqdump")
                nc.scalar.activation(sqd, gld[:, t, :], AF.Square,
                                     accum_out=g2c[:, t:t + 1])
            for t in range(m_tiles):
                nc.sync.dma_start(out=g2row_f[0:1, t * P:(t + 1) * P],
                                  in_=g2c[:, t:t + 1])
            nc.vector.tensor_copy(out=g2row, in_=g2row_f)

            # ---- main loop: one 128-row block of x at a time ----
            for i in range(n_chunks):
                gi, qi = divmod(i, XG)
                if qi == 0:
                    xld = sbuf.tile([P, XG, D], F32, tag="xload")
                    nc.sync.dma_start(
                        out=xld,
                        in_=x[gi * XG * P:(gi + 1) * XG * P, :]
                            .rearrange("(q p) d -> p q d", p=P),
                    )
                pt = tpsum.tile([P, P], F32, tag="tp")
                nc.tensor.transpose(pt, xld[:, qi, :], ident)
                nc.vector.tensor_scalar_mul(xTm2[:, i * P:(i + 1) * P], pt, -2.0)
                sqd = sbuf.tile([P, D], F32, tag="sqdump")
                nc.scalar.activation(sqd, xld[:, qi, :], AF.Square,
                                     accum_out=x2c[:, i:i + 1])

                stage = stagep.tile([P, M], F32, tag="stage")
                for j in range(j_tiles):
                    pm = mmpsum.tile([P, JT], F32, tag="mm")
                    nc.tensor.matmul(
                        pm,
                        lhsT=xTm2[:, i * P:(i + 1) * P],
                        rhs=gT[:, j * JT:(j + 1) * JT],
                        start=True, stop=False,
                    )
                    nc.tensor.matmul(
                        pm,
                        lhsT=ones1,
                        rhs=g2row[:, j * JT:(j + 1) * JT],
                        start=False, stop=True,
                    )
                    nc.scalar.activation(stage[:, j * JT:(j + 1) * JT], pm,
                                         AF.Sqrt, bias=x2c[:, i:i + 1])
                nc.vector.tensor_scalar_mul(stage, stage, -1.0)
                nc.sync.dma_start(out=out[i * P:(i + 1) * P, :], in_=stage)
    nc.finalize()
    return nc


def _get_nc():
    global _nc_cache
    if _nc_cache is None:
        _nc_cache = _build()
    return _nc_cache


def run_spmd(x_np, g_np, trace=False):
    """Run on 8 cores; returns (list of per-core out blocks, BassKernelResults)."""
    from concourse.bass_utils import run_bass_kernel_spmd

    in_maps = [
        {"x": x_np, "g": g_np[c * M:(c + 1) * M]} for c in range(NCORES)
    ]
    res = run_bass_kernel_spmd(_get_nc(), in_maps,
                               core_ids=list(range(NCORES)), trace=trace)
    blocks = [res.results[c]["out"] for c in range(NCORES)]
    return blocks, res


def kernel(image_features, gts):
    x_np = np.ascontiguousarray(np.asarray(image_features, dtype=np.float32))
    g_np = np.ascontiguousarray(np.asarray(gts, dtype=np.float32))
    assert x_np.shape == (N, D) and g_np.shape == (M_FULL, D)
    blocks, _ = run_spmd(x_np, g_np, trace=False)
    return np.concatenate(blocks, axis=1)
